# revision 1
# baseline (speedup 1.0000x reference)
"""Trainium2 Bass kernel for NSHE-style GNN message passing.

  enc = relu(concat(feat_a@W_a+b_a, feat_b@W_b+b_b, feat_c@W_c+b_c))
  support = enc @ gcn_W + gcn_b
  msg = support[edge_src] * edge_weight[:, None]
  com = segment_sum(msg, edge_dst, N);  out = l2_normalize(com, axis=1)

Distribution (8 NeuronCores, one shared SPMD NEFF), v3 (bf16 pipeline):
  - nodes are permuted into 8 balanced per-core slices (each slice mixes the
    three feature types); core k computes `support` rows for its slice in
    bf16, writing four DRAM segments (a rows 0-6249, a rows 6250-12499, b, c).
  - four pipelined AllGathers (one per segment) replicate the support table
    in bf16; phase 2's per-window work starts as soon as its table arrives.
  - each table is addressed as PAIRS of rows (256 B = the SWDGE gather
    granularity), so a src gather fetches [support[2s], support[2s+1]].
    All four windows are <=32767 pairs, fitting int16 gather indices.
  - edges are partitioned by destination slice; dst nodes are packed into
    196 blocks of 128 accumulator slots by a greedy degree balancer (the
    dst-slot map is decoupled from the phase-1 row map), and the edge
    stream is sorted by (psum-group, src-window, dst-block) and padded so
    tile counts are identical on every core (one program for all cores).
  - segment-sum on the tensor engine: per 128-edge tile a bf16 selection
    matrix M[e, j] = (j == dst_rel[e] + 128*half[e]) * w_e is built with one
    DVE tensor_scalar (4x mode); M's two halves then select the correct
    pair-half via two accumulating matmuls:
      psum[dst, :] += M[:, 0:128].T @ gt[:, 0:64]    (edges with even src)
      psum[dst, :] += M[:, 128:256].T @ gt[:, 64:128] (odd src)
  - l2 normalization is one batched pass; the host undoes the permutation.
"""

import numpy as np

N_A, N_B, N_C = 100000, 60000, 40000
D = 64
D_IN = (512, 256, 128)
NCORES = 8

P = 128                  # partitions / edge-tile size
BLK = 128                # dst span covered by one tile (one psum block)
BLKS_PER_GROUP = 64      # 8 psum banks x 8 blocks per bank
NSW = 4                  # src windows: a-lo, a-hi, b, c
GATHER_CHUNK = 1024      # idxs per dma_gather (SWDGE ring limit)
NQ = 4                   # SWDGE queues
DMA_SCRATCH = 16384      # SWDGE ring bytes/partition (1024-desc rings)
A_SPLIT = 6250           # a rows below this go to window 0, rest window 1

# Ablation switches for local profiling only (default off; harness sees full
# kernel). Set by probe scripts before _build_nc.
_ABLATE_GATHER = False   # replace dma_gather with bulk copy
_ABLATE_AG = False       # skip the AllGather collectives
_ABLATE_P2 = False       # skip phase-2 edge loop entirely


class _Plan:
    pass


def _make_plan(edge_src, edge_dst, edge_weight):
    """Host-side sharding: node permutation, uniform per-core edge schedule,
    operand arrays. Index manipulation only -- all float math runs on device
    (edge weights are moved, never combined, here)."""
    pl = _Plan()
    N = N_A + N_B + N_C
    SLICE = N // NCORES
    a_s, b_s, c_s = N_A // NCORES, N_B // NCORES, N_C // NCORES

    # phase-1 row map: node -> (core, prow); rows [0,a_s) type a etc.
    node_to_table = np.empty(N, dtype=np.int64)
    karr = np.arange(NCORES)
    for cnt, node0, off in ((a_s, 0, 0), (b_s, N_A, a_s), (c_s, N_A + N_B, a_s + b_s)):
        idx = node0 + (karr[:, None] * cnt + np.arange(cnt)[None, :])
        rows = SLICE * karr[:, None] + off + np.arange(cnt)[None, :]
        node_to_table[idx.ravel()] = rows.ravel()
    pl.N, pl.SLICE = N, SLICE
    pl.a_s, pl.b_s, pl.c_s = a_s, b_s, c_s
    pl.node_to_table = node_to_table

    # src mapping: (core, prow) -> window, pair index within window, half
    src_t = node_to_table[edge_src]
    c_of = src_t // SLICE
    r_of = src_t - c_of * SLICE
    is_a = r_of < a_s
    is_b = (~is_a) & (r_of < a_s + b_s)
    # position within the window's concatenated (core-major) row block
    pos = np.where(
        is_a,
        np.where(r_of < A_SPLIT, c_of * A_SPLIT + r_of,
                 c_of * (a_s - A_SPLIT) + (r_of - A_SPLIT)),
        np.where(is_b, c_of * b_s + (r_of - a_s),
                 c_of * c_s + (r_of - a_s - b_s)))
    win = np.where(is_a, (r_of >= A_SPLIT).astype(np.int64),
                   np.where(is_b, 2, 3)).astype(np.int64)
    pair_in_win = (pos >> 1).astype(np.int64)
    half = (pos & 1).astype(np.int64)
    pl.win_rows = [A_SPLIT, a_s - A_SPLIT, b_s, c_s]       # per-core rows
    pl.win_pairs = [w * NCORES // 2 for w in pl.win_rows]
    assert all(wp <= 32767 for wp in pl.win_pairs)

    # dst-slot map: greedy degree balancer packs each core's 25000 dsts into
    # 196 blocks of 128 slots, flattening per-(block, window) edge counts.
    dst_t = node_to_table[edge_dst]
    core = dst_t // SLICE
    r = dst_t - core * SLICE
    NBLK = (SLICE + BLK - 1) // BLK
    pl.NBLK = NBLK
    deg = np.zeros((NCORES, SLICE, NSW), np.int32)
    np.add.at(deg, (core, r, win), 1)
    tot = deg.sum(axis=2)
    slot_of_r = np.empty((NCORES, SLICE), np.int64)
    dst_slot_to_node = np.full((NCORES, NBLK * BLK), -1, np.int64)
    for c in range(NCORES):
        order = np.argsort(-tot[c], kind="stable")
        sums = np.zeros((NBLK, NSW), np.int64)
        cnt = np.zeros(NBLK, np.int64)
        d_all = deg[c, order].astype(np.int64)
        for i in range(SLICE):
            d = d_all[i]
            cand = np.flatnonzero(cnt < BLK)
            j = cand[np.argmin((sums[cand] + d).max(axis=1))]
            slot_of_r[c, order[i]] = j * BLK + cnt[j]
            sums[j] += d
            cnt[j] += 1
    # node lookup for unshard
    for c in range(NCORES):
        nodes_c = np.flatnonzero((node_to_table // SLICE) == c)
        rr = node_to_table[nodes_c] - c * SLICE
        dst_slot_to_node[c, slot_of_r[c, rr]] = nodes_c
    pl.dst_slot_to_node = dst_slot_to_node

    dst_slot = slot_of_r[core, r]
    blk = dst_slot // BLK
    NG = (NBLK + BLKS_PER_GROUP - 1) // BLKS_PER_GROUP
    pl.NG = NG
    pl.blocks_in_group = [min(BLKS_PER_GROUP, NBLK - g * BLKS_PER_GROUP)
                          for g in range(NG)]
    g_of = blk // BLKS_PER_GROUP
    bg_of = blk % BLKS_PER_GROUP

    cell = ((core * NG + g_of) * NSW + win) * BLKS_PER_GROUP + bg_of
    counts = np.bincount(cell, minlength=NCORES * NG * NSW * BLKS_PER_GROUP)
    counts = counts.reshape(NCORES, NG, NSW, BLKS_PER_GROUP)
    T = np.maximum(1, -(-counts.max(axis=0) // P))          # [NG, NSW, BPG]
    for g in range(NG):
        T[g, :, pl.blocks_in_group[g]:] = 0
    pl.T = T
    T_tot = int(T.sum())
    S_tot = T_tot * P
    pl.T_tot, pl.S_tot = T_tot, S_tot

    cell_sizes = (T * P).ravel()
    cb = np.zeros(len(cell_sizes) + 1, dtype=np.int64)
    np.cumsum(cell_sizes, out=cb[1:])
    pl.cell_base = cb[:-1].reshape(NG, NSW, BLKS_PER_GROUP)

    gidx = np.zeros((NCORES, S_tot), dtype=np.int16)
    dstrel = np.full((NCORES, S_tot), -999.0, dtype=np.float32)
    wstream = np.zeros((NCORES, S_tot), dtype=np.float32)

    order = np.lexsort((dst_slot, bg_of, win, g_of, core))
    srt_core = core[order]
    srt_cic = (g_of[order] * NSW + win[order]) * BLKS_PER_GROUP + bg_of[order]
    srt_src_rel = pair_in_win[order].astype(np.int16)
    srt_dst_rel = (dst_slot[order] - blk[order] * BLK
                   + half[order] * 128).astype(np.float32)
    srt_w = edge_weight[order].astype(np.float32)

    flat_base = pl.cell_base.ravel()
    ncell_pc = NG * NSW * BLKS_PER_GROUP
    for c in range(NCORES):
        m = srt_core == c
        cic = srt_cic[m]
        oc = np.bincount(cic, minlength=ncell_pc)
        within = (np.arange(len(cic)) - np.repeat(np.concatenate([[0], np.cumsum(oc)[:-1]]), oc))
        pos = flat_base[cic] + within
        gidx[c, pos] = srt_src_rel[m]
        dstrel[c, pos] = srt_dst_rel[m]
        wstream[c, pos] = srt_w[m]

    pl.dstloc = dstrel.reshape(NCORES, T_tot, P).transpose(0, 2, 1).copy()
    pl.wcol = wstream.reshape(NCORES, T_tot, P).transpose(0, 2, 1).copy()
    wrapped = gidx.reshape(NCORES, S_tot // 16, 16).transpose(0, 2, 1)
    pl.gidx_wrapped = np.ascontiguousarray(np.tile(wrapped, (1, 8, 1)))
    return pl


def _build_nc(pl):
    import concourse.bacc as bacc
    import concourse.mybir as mybir
    import concourse.tile as tile
    from concourse.bass import AP

    dt = mybir.dt
    Alu = mybir.AluOpType
    NG, SLICE, T = pl.NG, pl.SLICE, pl.T
    T_tot, S_tot, N = pl.T_tot, pl.S_tot, pl.N
    ACC_COLS = pl.NBLK * D

    nc = bacc.Bacc(None, target_bir_lowering=False, num_swdge_queues=NQ,
                   dynamic_dma_scratch_size=DMA_SCRATCH)

    def pad512(x):
        return ((x + 511) // 512) * 512

    featT = [nc.dram_tensor(f"featT_{t}", [D_IN[i], pad512(s)], dt.bfloat16,
                            kind="ExternalInput")
             for i, (t, s) in enumerate(zip("abc", (pl.a_s, pl.b_s, pl.c_s)))]
    Waug = [nc.dram_tensor(f"Waug_{t}", [D_IN[i], 65], dt.bfloat16,
                           kind="ExternalInput") for i, t in enumerate("abc")]
    baug = [nc.dram_tensor(f"baug_{t}", [1, 65], dt.bfloat16,
                           kind="ExternalInput") for t in "abc"]
    Wg_aug = nc.dram_tensor("Wg_aug", [65, D], dt.bfloat16, kind="ExternalInput")
    ones_rhs = nc.dram_tensor("ones_rhs", [1, 512], dt.bfloat16, kind="ExternalInput")
    iota2 = nc.dram_tensor("iota2", [P, 2 * P], dt.bfloat16, kind="ExternalInput")
    gidx_d = nc.dram_tensor("gidx", [P, S_tot // 16], dt.int16, kind="ExternalInput")
    # scalar-pointer operands (per-partition scalars) must be float32
    dstloc_d = nc.dram_tensor("dstloc", [P, T_tot], dt.float32, kind="ExternalInput")
    wcol_d = nc.dram_tensor("wcol", [P, T_tot], dt.float32, kind="ExternalInput")
    acc_out = nc.dram_tensor("acc_out", [P, ACC_COLS], dt.float32, kind="ExternalOutput")

    # per-window support slices (this core) and gathered tables (all cores),
    # bf16; tables addressed as pairs of rows (256B gather granularity)
    wnames = ("alo", "ahi", "b", "c")
    slice_w = [nc.dram_tensor(f"slice_{t}", [rows, D], dt.bfloat16)
               for t, rows in zip(wnames, pl.win_rows)]
    table_w = [nc.dram_tensor(f"table_{t}", [pairs, 2 * D], dt.bfloat16,
                              addr_space="Shared")
               for t, pairs in zip(wnames, pl.win_pairs)]

    with tile.TileContext(nc) as tc:
        with tc.tile_pool(name="const", bufs=1) as cpool:
            iota_sb = cpool.tile([P, 2 * P], dt.bfloat16)
            nc.sync.dma_start(out=iota_sb[:], in_=iota2[:])
            ones_sb = cpool.tile([1, 512], dt.bfloat16)
            nc.sync.dma_start(out=ones_sb[:], in_=ones_rhs[:])
            wg_sb = cpool.tile([65, D], dt.bfloat16)
            nc.sync.dma_start(out=wg_sb[:], in_=Wg_aug[:])

            # phase-1 row ranges of each window within this core's slice
            win_lo = [0, A_SPLIT, pl.a_s, pl.a_s + pl.b_s]
            win_hi = [A_SPLIT, pl.a_s, pl.a_s + pl.b_s, SLICE]

            def route_rows(sup_sb, qcol, sl0, v):
                """DMA sup_sb[p, qcol*D:...] holding slice-local rows
                [sl0, sl0+v) (p = row - sl0) to the per-window slice
                tensors, splitting at window bounds."""
                p = 0
                while v > 0:
                    sl = sl0 + p
                    w = next(i for i in range(NSW) if sl < win_hi[i])
                    seg = min(v, win_hi[w] - sl)
                    nc.sync.dma_start(
                        out=slice_w[w][sl - win_lo[w]:sl - win_lo[w] + seg, :],
                        in_=sup_sb[p:p + seg, qcol * D:(qcol + 1) * D])
                    p += seg
                    v -= seg

            # ============== phase 1: support slice =====================
            with (
                tc.tile_pool(name="p1w", bufs=1) as p1w,
                tc.tile_pool(name="p1f", bufs=3) as p1f,
                tc.tile_pool(name="p1s", bufs=3) as p1s,
                tc.tile_pool(name="p1o", bufs=3) as p1o,
                tc.tile_pool(name="psum1", bufs=2, space="PSUM") as psum1,
                tc.tile_pool(name="psum2", bufs=2, space="PSUM") as psum2,
            ):
                type_base = [0, pl.a_s, pl.a_s + pl.b_s]
                ag_fired = [False] * NSW
                for i, rows in enumerate((pl.a_s, pl.b_s, pl.c_s)):
                    din = D_IN[i]
                    nk = din // P
                    waug_sb = [p1w.tile([P, 65], dt.bfloat16, tag=f"waug{i}_{kk}",
                                        name=f"waug{i}_{kk}")
                               for kk in range(nk)]
                    for kk in range(nk):
                        nc.sync.dma_start(out=waug_sb[kk][:],
                                          in_=Waug[i][kk * P:(kk + 1) * P, :])
                    baug_sb = p1w.tile([1, 65], dt.bfloat16, tag=f"baug{i}")
                    nc.sync.dma_start(out=baug_sb[:], in_=baug[i][:])
                    for j in range((rows + 511) // 512):
                        c0 = j * 512
                        fts = []
                        for kk in range(nk):
                            ft = p1f.tile([P, 512], dt.bfloat16, tag="ft")
                            nc.sync.dma_start(
                                out=ft[:],
                                in_=featT[i][kk * P:(kk + 1) * P, c0:c0 + 512])
                            fts.append(ft)
                        encT_ps = psum1.tile([65, 512], dt.float32, tag="encT")
                        for kk in range(nk):
                            nc.tensor.matmul(
                                out=encT_ps[:],
                                lhsT=waug_sb[kk][:],
                                rhs=fts[kk][:],
                                start=(kk == 0), stop=False, skip_group_check=True)
                        nc.tensor.matmul(out=encT_ps[:], lhsT=baug_sb[:],
                                         rhs=ones_sb[:], start=False, stop=True,
                                         skip_group_check=True)
                        encT_sb = p1s.tile([65, 512], dt.bfloat16, tag="encT_sb")
                        nc.vector.tensor_scalar_max(encT_sb[:], encT_ps[:], 0.0)
                        sup_ps = psum2.tile([P, 256], dt.float32, tag="sup")
                        # start=True zeroes the whole PSUM bank, so only the
                        # first matmul of the bank sets it.
                        for q in range(4):
                            nc.tensor.matmul(
                                out=sup_ps[:, q * D:(q + 1) * D],
                                lhsT=encT_sb[:, q * P:(q + 1) * P], rhs=wg_sb[:],
                                start=(q == 0), stop=(q == 3),
                                skip_group_check=True)
                        sup_sb = p1o.tile([P, 256], dt.bfloat16, tag="sup_sb")
                        nc.vector.tensor_copy(out=sup_sb[:], in_=sup_ps[:])
                        for q in range(4):
                            r0 = c0 + q * P
                            v = min(P, rows - r0)
                            if v <= 0:
                                break
                            route_rows(sup_sb, q, type_base[i] + r0, v)
                        # allgathers fire as soon as their rows are written
                        covered = type_base[i] + min(rows, c0 + 512)
                        if not _ABLATE_AG:
                            for w in range(NSW):
                                if not ag_fired[w] and win_hi[w] <= covered:
                                    ag_fired[w] = True
                                    nc.gpsimd.collective_compute(
                                        "AllGather", Alu.bypass,
                                        replica_groups=[list(range(NCORES))],
                                        ins=[slice_w[w][:]], outs=[table_w[w][:]])

            # ============== phase 2: gather + segment matmul ===========
            with (
                tc.tile_pool(name="acc", bufs=1) as accpool,
                tc.tile_pool(name="gpool", bufs=6) as gpool,
                tc.tile_pool(name="ipool", bufs=3) as ipool,
                tc.tile_pool(name="opool", bufs=2) as opool,
                tc.tile_pool(name="mpool", bufs=6) as mpool,
                tc.tile_pool(name="psacc", bufs=1, space="PSUM") as psacc,
            ):
                acc_sb = accpool.tile([P, ACC_COLS], dt.float32)
                if _ABLATE_P2:
                    nc.vector.memset(acc_sb[:], 0.0)

                # per-tile scalars for the whole edge schedule stay resident
                dl_sb = opool.tile([P, T_tot], dt.float32, tag="dl")
                nc.sync.dma_start(out=dl_sb[:], in_=dstloc_d[:])
                wc_sb = opool.tile([P, T_tot], dt.float32, tag="wc")
                nc.sync.dma_start(out=wc_sb[:], in_=wcol_d[:])

                gctr = 0
                # window-outer: each window's work starts right after its
                # AllGather; per (window, group) psum pass add-flushes into acc
                for sw in range(NSW if not _ABLATE_P2 else 0):
                    tbl = table_w[sw]
                    pcnt = pl.win_pairs[sw]
                    for g in range(NG):
                        nblk_g = pl.blocks_in_group[g]
                        Tsw = int(T[g, sw].sum())
                        s0 = int(pl.cell_base[g, sw, 0])
                        col0 = int(T[:g].sum()) + int(T[g, :sw].sum())
                        nslots = Tsw * P
                        it = ipool.tile([P, nslots // 16], dt.int16, tag="idx")
                        nc.sync.dma_start(
                            out=it[:], in_=gidx_d[:, s0 // 16:(s0 + nslots) // 16])
                        chunks = []
                        off = 0
                        while off < nslots:
                            ln = min(GATHER_CHUNK, nslots - off)
                            gt = gpool.tile([P, GATHER_CHUNK // P, 2 * D],
                                            dt.bfloat16, tag="gat")
                            if not _ABLATE_GATHER:
                                nc.gpsimd.dma_gather(
                                    out_ap=gt[:, :ln // P, :],
                                    in_ap=tbl[0:pcnt, :],
                                    idxs_ap=it[:, off // 16:(off + ln) // 16],
                                    num_idxs=ln, num_idxs_reg=ln, elem_size=2 * D,
                                    queue_num=gctr % NQ)
                            else:
                                nc.sync.dma_start(
                                    out=gt[:, :ln // P, :],
                                    in_=tbl[0:ln, :]
                                    .rearrange("(p k) d -> p k d", p=P))
                            gctr += 1
                            chunks.append(gt)
                            off += ln
                        banks = [psacc.tile([P, 512], dt.float32, tag=f"bank{b}",
                                           name=f"bank_s{sw}g{g}_{b}")
                                 for b in range(8)]
                        bank_first = [True] * 8  # start=True zeroes the bank
                        last_bg_of_bank = {}
                        for bg in range(nblk_g):
                            last_bg_of_bank[bg // 8] = bg
                        tloc = 0
                        for bg in range(BLKS_PER_GROUP):
                            nt = int(T[g, sw, bg])
                            if nt == 0:
                                continue
                            bank, boff = bg // 8, (bg % 8) * D
                            for t in range(nt):
                                slot0 = (tloc + t) * P
                                gt = chunks[slot0 // GATHER_CHUNK]
                                kk = (slot0 % GATHER_CHUNK) // P
                                col = col0 + tloc + t
                                m = mpool.tile([P, 2 * P], dt.bfloat16, tag="m")
                                nc.vector.tensor_scalar(
                                    out=m[:], in0=iota_sb[:],
                                    scalar1=dl_sb[:, col:col + 1],
                                    scalar2=wc_sb[:, col:col + 1],
                                    op0=Alu.is_equal, op1=Alu.mult)
                                last = (bg == last_bg_of_bank[bank] and
                                        t == nt - 1)
                                nc.tensor.matmul(
                                    out=banks[bank][:, boff:boff + D],
                                    lhsT=m[:, 0:P],
                                    rhs=gt[:, kk, 0:D],
                                    start=bank_first[bank], stop=False,
                                    skip_group_check=True)
                                bank_first[bank] = False
                                nc.tensor.matmul(
                                    out=banks[bank][:, boff:boff + D],
                                    lhsT=m[:, P:2 * P],
                                    rhs=gt[:, kk, D:2 * D],
                                    start=False, stop=last,
                                    skip_group_check=True)
                            tloc += nt
                        # flush this window's partial sums into the accumulator
                        ncols = nblk_g * D
                        for b in range(8):
                            bc = min(512, max(0, ncols - b * 512))
                            if bc == 0:
                                break
                            a0 = g * BLKS_PER_GROUP * D + b * 512
                            if sw == 0:
                                nc.vector.tensor_copy(
                                    out=acc_sb[:, a0:a0 + bc],
                                    in_=banks[b][:, :bc])
                            else:
                                nc.vector.tensor_tensor(
                                    out=acc_sb[:, a0:a0 + bc],
                                    in0=acc_sb[:, a0:a0 + bc],
                                    in1=banks[b][:, :bc], op=Alu.add)

                # ============== phase 3: normalize + out ===============
                with tc.tile_pool(name="npool", bufs=1) as npool:
                    NBLKn = ACC_COLS // D
                    sq = npool.tile([P, ACC_COLS], dt.float32, tag="sq")
                    nc.vector.tensor_tensor(out=sq[:], in0=acc_sb[:], in1=acc_sb[:],
                                            op=Alu.mult)
                    ss = npool.tile([P, NBLKn], dt.float32, tag="ss")
                    nc.vector.tensor_reduce(
                        out=ss[:],
                        in_=sq[:].rearrange("p (b d) -> p b d", d=D),
                        axis=mybir.AxisListType.X, op=Alu.add)
                    nrm = npool.tile([P, NBLKn], dt.float32, tag="nrm")
                    nc.scalar.activation(nrm[:], ss[:],
                                         mybir.ActivationFunctionType.Sqrt)
                    nc.vector.tensor_scalar_max(nrm[:], nrm[:], 1e-12)
                    rec = npool.tile([P, NBLKn], dt.float32, tag="rec")
                    nc.vector.reciprocal(rec[:], nrm[:])
                    rap = rec[:]
                    rec_b = AP(rap.tensor, rap.offset, list(rap.ap) + [[0, D]])
                    nc.vector.tensor_tensor(
                        out=acc_sb[:].rearrange("p (b d) -> p b d", d=D),
                        in0=acc_sb[:].rearrange("p (b d) -> p b d", d=D),
                        in1=rec_b, op=Alu.mult)
                    nc.sync.dma_start(out=acc_out[:], in_=acc_sb[:])

    nc.compile()
    return nc


def _in_maps(pl, inputs):
    import ml_dtypes
    bf16 = ml_dtypes.bfloat16

    feats = [np.asarray(inputs["feat_a"], np.float32),
             np.asarray(inputs["feat_b"], np.float32),
             np.asarray(inputs["feat_c"], np.float32)]
    Ws = [np.asarray(inputs["W_a"], np.float32),
          np.asarray(inputs["W_b"], np.float32),
          np.asarray(inputs["W_c"], np.float32)]
    bs = [np.asarray(inputs["b_a"], np.float32),
          np.asarray(inputs["b_b"], np.float32),
          np.asarray(inputs["b_c"], np.float32)]
    gcn_W = np.asarray(inputs["gcn_W"], np.float32)
    gcn_b = np.asarray(inputs["gcn_b"], np.float32)

    Waug = [np.ascontiguousarray(
        np.concatenate([W, np.zeros((W.shape[0], 1), np.float32)], 1)).astype(bf16)
        for W in Ws]
    baug = [np.ascontiguousarray(
        np.concatenate([b, np.ones(1, np.float32)])[None, :]).astype(bf16)
        for b in bs]
    Wg_aug = np.ascontiguousarray(
        np.concatenate([gcn_W, gcn_b[None, :]], 0)).astype(bf16)
    ones = np.ones((1, 512), bf16)
    iota2 = np.ascontiguousarray(
        np.tile(np.arange(2 * P, dtype=np.float32)[None, :], (P, 1))).astype(bf16)

    sizes = [pl.a_s, pl.b_s, pl.c_s]
    maps = []
    for c in range(NCORES):
        m = {}
        for t, nm in enumerate("abc"):
            cnt = sizes[t]
            rows = feats[t][c * cnt:(c + 1) * cnt]
            padcols = ((cnt + 511) // 512) * 512
            ft = np.zeros((D_IN[t], padcols), np.float32)
            ft[:, :cnt] = rows.T
            m[f"featT_{nm}"] = ft.astype(bf16)
            m[f"Waug_{nm}"] = Waug[t]
            m[f"baug_{nm}"] = baug[t]
        m["Wg_aug"] = Wg_aug
        m["ones_rhs"] = ones
        m["iota2"] = iota2
        m["gidx"] = pl.gidx_wrapped[c]
        m["dstloc"] = pl.dstloc[c]
        m["wcol"] = pl.wcol[c]
        maps.append(m)
    return maps


def _unshard(pl, results):
    out = np.zeros((pl.N, D), np.float32)
    for c in range(NCORES):
        acc = results[c]["acc_out"]
        blk = acc.reshape(P, pl.NBLK, D)
        nodes = pl.dst_slot_to_node[c]
        for b in range(pl.NBLK):
            sl = nodes[b * BLK:(b + 1) * BLK]
            valid = sl >= 0
            out[sl[valid]] = blk[valid, b, :]
    return out


def kernel(**inputs):
    from concourse.bass_utils import run_bass_kernel_spmd

    edge_src = np.asarray(inputs["edge_src"]).astype(np.int64)
    edge_dst = np.asarray(inputs["edge_dst"]).astype(np.int64)
    edge_weight = np.asarray(inputs["edge_weight"], dtype=np.float32)

    pl = _make_plan(edge_src, edge_dst, edge_weight)
    nc = _build_nc(pl)
    maps = _in_maps(pl, inputs)
    res = run_bass_kernel_spmd(nc, maps, core_ids=list(range(NCORES)))
    return _unshard(pl, [res.results[c] for c in range(NCORES)])

